# revision 10
# baseline (speedup 1.0000x reference)
"""Trainium2 Bass kernel for nn_BbVertLoss (point-in-bbox CE + IoU + L2 loss).

Strategy (pure data parallel, hardcoded for B=16, N=40960, H=24, 8 cores):
  - Each core gets 2 batches. Points live on partitions: partition p holds
    640 consecutive points of batch p//64 (local), laid out [128, 640*9].
  - Math reformulation per (batch, box h, point):
      u_d   = (a_d - x_d)(x_d - b_d) = r_d^2 - (x_d - c_d)^2,  c=(a+b)/2, r=(b-a)/2
      min_d u_d = -max_d((x_d-c_d)^2 - r_d^2) = -w
      pred: p = sigmoid(clip(100*min_u, -20, 20)) = sigmoid(-100*clip(w, -0.2, 0.2))
      gt:   g = 1{min_d u'_d > 0} = 1{w' < 0}
      ce    = -g*log(p+eps) - (1-g)*log(1-p+eps) = -log(|p + (g-1)| + eps)
      TP    = sum p*g = (sum|p+(g-1)| + sum p + sum g - Npts) / 2
    Device reduces over points per (partition, h): S_p (ACT sigmoid accum),
    S_g (custom DVE accum), S_sel (custom DVE accum), S_ln (ACT Ln accum).
  - Custom fused DVE ops (registered at import into concourse.dve_ops):
      ANT_SUB2MAX:     max(in0-s0, in1-s1)
      ANT_SQMAXCLIP:   clip(max((in0-s0)^2-s1, in1), imm2, -imm2)
      ANT_SQMAXLT0SUM: (max((in0-s0)^2-s1, in1) < 0) [+ sum]
      ANT_ABSPG1SUM:   |in0 + (in1-1)| [+ sum]
  - Host: partition+core reduction and final combine in f64.
"""

import numpy as np

B, N, H = 16, 40960, 24
NCORES = 8
BPC = B // NCORES            # batches per core = 2
PPB = 64                     # partitions per batch
FPT = N // PPB               # points per partition = 640
NPART = BPC * PPB            # 128
RAWF = FPT * 3               # xyz de-interleaved on host: [x|y|z] per partition
GRP = 6                      # h-group size for ACT table-load amortization

_CACHE = {}


def _register_custom_ops():
    """Register fused DVE ops in the module-level registries (idempotent)."""
    import concourse.dve_ops as dops
    from concourse.dve_spec import (Spec, Src0, Src1, C0, C1, C2, Zero, One,
                                    maxx, minn, sq, lower, AluOp)
    from concourse.dve_table_gen import dve_ver_for
    from concourse.dve_uop import DveOpSpec

    if "ANT_SUB2MAX" in dops._SUB_OPCODE_FOR_NAME:
        return

    ver = dve_ver_for("TRN2")

    def ref_sub2max(in0, in1, s0, s1, imm2):
        return np.maximum(in0 - s0, in1 - s1)

    def ref_sqmaxclip(in0, in1, s0, s1, imm2):
        return np.minimum(np.maximum(np.maximum((in0 - s0) ** 2 - s1, in1),
                                     imm2), -imm2)

    def ref_sqmaxlt0sum(in0, in1, s0, s1, imm2):
        b = (np.maximum((in0 - s0) ** 2 - s1, in1) < 0.0).astype(np.float32)
        return b, b.reshape(b.shape[0], -1).sum(axis=-1, keepdims=True).astype(
            np.float32)

    def ref_abspg1sum(in0, in1, s0, s1, imm2):
        t = ((in1 - np.float32(1.0)) + in0).astype(np.float32)
        b = np.abs(t)
        return b, b.reshape(b.shape[0], -1).sum(axis=-1, keepdims=True).astype(
            np.float32)

    def ref_sqsubmax(in0, in1, s0, s1, imm2):
        return np.maximum((in0 - s0) ** 2 - s1, in1)

    _t = Src0 + (Src1 - One)
    specs = [
        ("ANT_SUB2MAX", Spec(body=maxx(Src0 - C0, Src1 - C1),
                             reference=ref_sub2max)),
        ("ANT_SQMAXCLIP", Spec(body=minn(maxx(maxx(sq(Src0 - C0) - C1, Src1),
                                              C2), Zero - C2),
                               reference=ref_sqmaxclip)),
        ("ANT_SQMAXLT0SUM", Spec(body=(maxx(sq(Src0 - C0) - C1, Src1) < Zero),
                                 accum=AluOp.ADD, reference=ref_sqmaxlt0sum)),
        ("ANT_ABSPG1SUM", Spec(body=maxx(_t, Zero - _t),
                               accum=AluOp.ADD, reference=ref_abspg1sum)),
        ("ANT_SQSUBMAX", Spec(body=maxx(sq(Src0 - C0) - C1, Src1),
                              reference=ref_sqsubmax)),
    ]
    for name, spec in specs:
        opcode = max(dops._SUB_OPCODE_FOR_NAME.values()) + 1
        assert opcode < 0x20
        tmp = DveOpSpec(name=name, opcode=opcode, uops=lower(spec, ver=ver),
                        rd1_en=True)
        op = dops.DveOp(name, spec, subdim=False, uops_sha={ver: tmp.sha(ver)})
        dops.OPS.append(op)
        dops.CUSTOM_DVE_SPECS[name] = spec
        dops._SUB_OPCODE_FOR_NAME[name] = opcode
    dops_by_name = {o.name: o for o in dops.OPS}
    _CACHE["ops"] = dops_by_name


def _build_module():
    import concourse.bacc as bacc
    import concourse.tile as tile
    from concourse import mybir

    _register_custom_ops()
    OPS = _CACHE["ops"]

    f32 = mybir.dt.float32
    Act = mybir.ActivationFunctionType

    nc = bacc.Bacc("TRN2", debug=False)

    xpc = nc.dram_tensor("xpc", [NPART, RAWF], f32, kind="ExternalInput")
    scal = nc.dram_tensor("scal", [NPART, H * 12], f32, kind="ExternalInput")
    accP_d = nc.dram_tensor("accP", [NPART, H], f32, kind="ExternalOutput")
    accG_d = nc.dram_tensor("accG", [NPART, H], f32, kind="ExternalOutput")
    accS_d = nc.dram_tensor("accS", [NPART, H], f32, kind="ExternalOutput")
    accL_d = nc.dram_tensor("accL", [NPART, H], f32, kind="ExternalOutput")

    Alu = mybir.AluOpType
    with tile.TileContext(nc) as tc:
        with (
            tc.tile_pool(name="data", bufs=1) as data,
            tc.tile_pool(name="work", bufs=3) as work,
            tc.tile_pool(name="phase", bufs=GRP + 2) as phase,
        ):
            raw = data.tile([NPART, RAWF], f32, tag="raw")
            for q in range(4):
                p0, p1 = 32 * q, 32 * (q + 1)
                nc.sync.dma_start(out=raw[p0:p1, :], in_=xpc[p0:p1, :])
            sc = data.tile([NPART, H * 12], f32, tag="sc")
            nc.sync.dma_start(out=sc[:], in_=scal[:])
            eps8 = data.tile([NPART, 1], f32, tag="eps8")
            nc.vector.memset(eps8[:], 1e-8)

            accP = data.tile([NPART, H], f32, tag="accP")
            accG = data.tile([NPART, H], f32, tag="accG")
            accS = data.tile([NPART, H], f32, tag="accS")
            accL = data.tile([NPART, H], f32, tag="accL")

            xs = [raw[:, FPT * d : FPT * (d + 1)] for d in range(3)]

            def col(h, j):
                return sc[:, 12 * h + j : 12 * h + j + 1]

            for h0 in range(0, H, GRP):
                hs = range(h0, min(h0 + GRP, H))
                tcls, gs, ps, sels = {}, {}, {}, {}
                for h in hs:
                    # pred: tcl = clip(max_d((x_d-c_d)^2 - r_d^2), +-0.2)
                    sqy = work.tile([NPART, FPT], f32, tag="sqy")
                    nc.scalar.activation(sqy[:], xs[1], Act.Square,
                                         bias=col(h, 0), scale=1.0)
                    qsy = work.tile([NPART, FPT], f32, tag="qsy")
                    nc.gpsimd.tensor_scalar(qsy[:], sqy[:], col(h, 1), None,
                                            op0=Alu.subtract)
                    m1 = work.tile([NPART, FPT], f32, tag="m1")
                    nc.vector._custom_dve(OPS["ANT_SQSUBMAX"], out=m1[:],
                                          in0=xs[2], in1=qsy[:],
                                          s0=col(h, 2), s1=col(h, 3))
                    tcl = phase.tile([NPART, FPT], f32, tag="tcl")
                    nc.vector._custom_dve(OPS["ANT_SQMAXCLIP"], out=tcl[:],
                                          in0=xs[0], in1=m1[:],
                                          s0=col(h, 4), s1=col(h, 5), imm2=-0.2)
                    tcls[h] = tcl

                    # gt: g = 1{max_d((x_d-c'_d)^2 - r'^2_d) < 0}
                    sqgy = work.tile([NPART, FPT], f32, tag="sqgy")
                    nc.scalar.activation(sqgy[:], xs[1], Act.Square,
                                         bias=col(h, 6), scale=1.0)
                    qsgy = work.tile([NPART, FPT], f32, tag="qsgy")
                    nc.gpsimd.tensor_scalar(qsgy[:], sqgy[:], col(h, 7), None,
                                            op0=Alu.subtract)
                    mg1 = work.tile([NPART, FPT], f32, tag="mg1")
                    nc.vector._custom_dve(OPS["ANT_SQSUBMAX"], out=mg1[:],
                                          in0=xs[2], in1=qsgy[:],
                                          s0=col(h, 8), s1=col(h, 9))
                    g = phase.tile([NPART, FPT], f32, tag="g")
                    nc.vector._custom_dve(OPS["ANT_SQMAXLT0SUM"], out=g[:],
                                          in0=xs[0], in1=mg1[:],
                                          s0=col(h, 10), s1=col(h, 11),
                                          accum_out=accG[:, h : h + 1])
                    gs[h] = g

                for h in hs:   # sigmoid phase (one ACT table load per group)
                    p = phase.tile([NPART, FPT], f32, tag="p")
                    nc.scalar.activation(p[:], tcls[h][:], Act.Sigmoid,
                                         bias=0.0, scale=-100.0,
                                         accum_out=accP[:, h : h + 1])
                    ps[h] = p
                for h in hs:   # sel = |p + (g-1)|
                    sel = phase.tile([NPART, FPT], f32, tag="sel")
                    nc.vector._custom_dve(OPS["ANT_ABSPG1SUM"], out=sel[:],
                                          in0=ps[h][:], in1=gs[h][:],
                                          accum_out=accS[:, h : h + 1])
                    sels[h] = sel
                for h in hs:   # Ln phase (one ACT table load per group)
                    lnsel = work.tile([NPART, FPT], f32, tag="lnsel")
                    nc.scalar.activation(lnsel[:], sels[h][:], Act.Ln,
                                         bias=eps8[:], scale=1.0,
                                         accum_out=accL[:, h : h + 1])

            nc.sync.dma_start(out=accP_d[:], in_=accP[:])
            nc.sync.dma_start(out=accG_d[:], in_=accG[:])
            nc.sync.dma_start(out=accS_d[:], in_=accS[:])
            nc.sync.dma_start(out=accL_d[:], in_=accL[:])

    nc.compile()
    return nc


def _get_module():
    if "nc" not in _CACHE:
        _CACHE["nc"] = _build_module()
    return _CACHE["nc"]


def _make_inputs(X_pc, y_bbvert_pred, Y_bbvert):
    """Build per-core input maps (host-side shard + scalar precompute)."""
    X_pc = np.ascontiguousarray(X_pc, dtype=np.float32)
    pred = np.asarray(y_bbvert_pred, dtype=np.float32)
    gt = np.asarray(Y_bbvert, dtype=np.float32)

    # columns per (B,H): [-c_y, rsq_y, c_z, rsq_z, c_x, rsq_x] pred then gt
    def params(t):
        a = t[:, :, 0, :]
        b = t[:, :, 1, :]
        c = ((a + b) * np.float32(0.5)).astype(np.float32)
        r = ((b - a) * np.float32(0.5)).astype(np.float32)
        rsq = (r * r).astype(np.float32)
        return np.stack([-c[:, :, 1], rsq[:, :, 1], c[:, :, 2], rsq[:, :, 2],
                         c[:, :, 0], rsq[:, :, 0]], axis=-1)

    sc_all = np.concatenate([params(pred), params(gt)], axis=-1)  # [B,H,12]

    in_maps = []
    for k in range(NCORES):
        rows = []
        scs = []
        for b in range(BPC):
            bi = BPC * k + b
            # de-interleave xyz on host: partition row = [x(640)|y(640)|z(640)]
            xyz = X_pc[bi].reshape(PPB, FPT, 9)[:, :, :3]
            rows.append(xyz.transpose(0, 2, 1).reshape(PPB, RAWF))
            scs.append(np.broadcast_to(sc_all[bi][None], (PPB, H, 12)))
        in_maps.append({
            "xpc": np.ascontiguousarray(np.concatenate(rows, axis=0)),
            "scal": np.ascontiguousarray(
                np.concatenate(scs, axis=0).reshape(NPART, H * 12)),
        })
    return in_maps


def _combine(results, y_bbvert_pred, Y_bbvert):
    """Host-side: partition+core reduction and final loss combine (f64)."""
    pred = np.asarray(y_bbvert_pred, dtype=np.float32)
    gt = np.asarray(Y_bbvert, dtype=np.float32)

    Sp = np.zeros((B, H)); Sg = np.zeros((B, H))
    Ss = np.zeros((B, H)); Sl = np.zeros((B, H))
    for k in range(NCORES):
        r = results[k]
        for b in range(BPC):
            bi = BPC * k + b
            s = slice(PPB * b, PPB * (b + 1))
            Sp[bi] = r["accP"][s].astype(np.float64).sum(axis=0)
            Sg[bi] = r["accG"][s].astype(np.float64).sum(axis=0)
            Ss[bi] = r["accS"][s].astype(np.float64).sum(axis=0)
            Sl[bi] = r["accL"][s].astype(np.float64).sum(axis=0)

    Tp = (Ss + Sg + Sp - float(N)) * 0.5
    helper = (gt.reshape(B, H, 6).sum(axis=-1) > 0.0).astype(np.float64)
    Sce = -Sl

    denom_ce = helper.sum() * N
    loss_ce = (Sce * helper).sum() / denom_ce

    iou_all = -(Tp / (Sp + Sg - Tp + 1e-6))
    loss_iou = (iou_all * helper).sum() / helper.sum()

    l2_all = ((gt.astype(np.float64) - pred.astype(np.float64)) ** 2
              ).reshape(B, H, 6).mean(axis=-1)
    l2_pos = (l2_all * helper).sum() / helper.sum()
    negw = (1.0 - helper)[:, :, None]
    dneg = (pred[:, :, 0, :].astype(np.float64) - pred[:, :, 1, :].astype(np.float64))
    l2_neg = ((negw * dneg) ** 2).sum() / ((1.0 - helper).sum() + 1e-8)
    loss_l2 = l2_pos + l2_neg

    total = loss_ce + loss_l2 + loss_iou
    return (np.float32(total), np.float32(loss_l2),
            np.float32(loss_ce), np.float32(loss_iou))


def run(X_pc, y_bbvert_pred, Y_bbvert, trace=False):
    from concourse.bass_utils import run_bass_kernel_spmd

    nc = _get_module()
    in_maps = _make_inputs(X_pc, y_bbvert_pred, Y_bbvert)
    res = run_bass_kernel_spmd(nc, in_maps, core_ids=list(range(NCORES)),
                               trace=trace)
    out = _combine(res.results, y_bbvert_pred, Y_bbvert)
    return out, res


def kernel(X_pc, y_bbvert_pred, Y_bbvert):
    out, _ = run(X_pc, y_bbvert_pred, Y_bbvert, trace=False)
    return out


# revision 16
# speedup vs baseline: 3.8517x; 3.8517x over previous
"""Trainium2 Bass kernel for nn_BbVertLoss (point-in-bbox CE + IoU + L2 loss).

Strategy (pure data parallel, hardcoded for B=16, N=40960, H=24, 8 cores):
  - Each core gets 2 batches. Points live on partitions: partition p holds
    640 consecutive points of batch p//64 (local), laid out [128, 640*9].
  - Math reformulation per (batch, box h, point):
      u_d   = (a_d - x_d)(x_d - b_d) = r_d^2 - (x_d - c_d)^2,  c=(a+b)/2, r=(b-a)/2
      min_d u_d = -max_d((x_d-c_d)^2 - r_d^2) = -w
      pred: p = sigmoid(clip(100*min_u, -20, 20)) = sigmoid(-100*clip(w, -0.2, 0.2))
      gt:   g = 1{min_d u'_d > 0} = 1{w' < 0}
      ce    = -g*log(p+eps) - (1-g)*log(1-p+eps) = -log(|p + (g-1)| + eps)
      TP    = sum p*g = (sum|p+(g-1)| + sum p + sum g - Npts) / 2
    Device reduces over points per (partition, h): S_p (ACT sigmoid accum),
    S_g (custom DVE accum), S_sel (custom DVE accum), S_ln (ACT Ln accum).
  - Custom fused DVE ops (registered at import into concourse.dve_ops):
      ANT_SUB2MAX:     max(in0-s0, in1-s1)
      ANT_SQMAXCLIP:   clip(max((in0-s0)^2-s1, in1), imm2, -imm2)
      ANT_SQMAXLT0SUM: (max((in0-s0)^2-s1, in1) < 0) [+ sum]
      ANT_ABSPG1SUM:   |in0 + (in1-1)| [+ sum]
  - Host: partition+core reduction and final combine in f64.
"""

import numpy as np

B, N, H = 16, 40960, 24
NCORES = 8
BPC = B // NCORES            # batches per core = 2
PPB = 64                     # partitions per batch
FPT = N // PPB               # points per partition = 640
NPART = BPC * PPB            # 128
RAWF = FPT * 3               # xyz de-interleaved on host: [x|y|z] per partition
GRP = 8                      # h-group size for ACT table-load amortization
SCW = 16                     # scal columns per (batch, h)

_CACHE = {}


def _register_custom_ops():
    """Register fused DVE ops in the module-level registries (idempotent)."""
    import concourse.dve_ops as dops
    from concourse.dve_spec import (Spec, Src0, Src1, C0, C1, C2, Zero, One,
                                    maxx, minn, sq, lower, AluOp)
    from concourse.dve_table_gen import dve_ver_for
    from concourse.dve_uop import DveOpSpec

    if "ANT_SUB2MAX" in dops._SUB_OPCODE_FOR_NAME:
        return

    ver = dve_ver_for("TRN2")

    def ref_sub2max(in0, in1, s0, s1, imm2):
        return np.maximum(in0 - s0, in1 - s1)

    def ref_sqmaxclip(in0, in1, s0, s1, imm2):
        return np.minimum(np.maximum(np.maximum((in0 - s0) ** 2 - s1, in1),
                                     imm2), -imm2)

    def ref_sqmaxlt0sum(in0, in1, s0, s1, imm2):
        b = (np.maximum((in0 - s0) ** 2 - s1, in1) < 0.0).astype(np.float32)
        return b, b.reshape(b.shape[0], -1).sum(axis=-1, keepdims=True).astype(
            np.float32)

    def ref_abspg1sum(in0, in1, s0, s1, imm2):
        t = ((in1 - np.float32(1.0)) + in0).astype(np.float32)
        b = np.abs(t)
        return b, b.reshape(b.shape[0], -1).sum(axis=-1, keepdims=True).astype(
            np.float32)

    def ref_sqsubmax(in0, in1, s0, s1, imm2):
        return np.maximum((in0 - s0) ** 2 - s1, in1)

    _t = Src0 + (Src1 - One)
    specs = [
        ("ANT_SUB2MAX", Spec(body=maxx(Src0 - C0, Src1 - C1),
                             reference=ref_sub2max)),
        ("ANT_SQMAXCLIP", Spec(body=minn(maxx(maxx(sq(Src0 - C0) - C1, Src1),
                                              C2), Zero - C2),
                               reference=ref_sqmaxclip)),
        ("ANT_SQMAXLT0SUM", Spec(body=(maxx(sq(Src0 - C0) - C1, Src1) < Zero),
                                 accum=AluOp.ADD, reference=ref_sqmaxlt0sum)),
        ("ANT_ABSPG1SUM", Spec(body=maxx(_t, Zero - _t),
                               accum=AluOp.ADD, reference=ref_abspg1sum)),
        ("ANT_SQSUBMAX", Spec(body=maxx(sq(Src0 - C0) - C1, Src1),
                              reference=ref_sqsubmax)),
        ("ANT_SQSUB", Spec(body=sq(Src0 - C0) - C1,
                           reference=lambda in0, in1, s0, s1, imm2:
                               (in0 - s0) ** 2 - s1)),
    ]
    for name, spec in specs:
        opcode = max(dops._SUB_OPCODE_FOR_NAME.values()) + 1
        assert opcode < 0x20
        tmp = DveOpSpec(name=name, opcode=opcode, uops=lower(spec, ver=ver),
                        rd1_en=True)
        op = dops.DveOp(name, spec, subdim=False, uops_sha={ver: tmp.sha(ver)})
        dops.OPS.append(op)
        dops.CUSTOM_DVE_SPECS[name] = spec
        dops._SUB_OPCODE_FOR_NAME[name] = opcode
    dops_by_name = {o.name: o for o in dops.OPS}
    _CACHE["ops"] = dops_by_name


def _build_module():
    import concourse.bacc as bacc
    import concourse.tile as tile
    from concourse import mybir

    _register_custom_ops()
    OPS = _CACHE["ops"]

    f32 = mybir.dt.float32
    Act = mybir.ActivationFunctionType

    nc = bacc.Bacc("TRN2", debug=False)

    xpc = nc.dram_tensor("xpc", [NPART, RAWF], f32, kind="ExternalInput")
    scal = nc.dram_tensor("scal", [NPART, H * SCW], f32, kind="ExternalInput")
    accP_d = nc.dram_tensor("accP", [NPART, H], f32, kind="ExternalOutput")
    accG_d = nc.dram_tensor("accG", [NPART, H], f32, kind="ExternalOutput")
    accS_d = nc.dram_tensor("accS", [NPART, H], f32, kind="ExternalOutput")
    accL_d = nc.dram_tensor("accL", [NPART, H], f32, kind="ExternalOutput")

    Alu = mybir.AluOpType
    with tile.TileContext(nc) as tc:
        with (
            tc.tile_pool(name="data", bufs=1) as data,
            tc.tile_pool(name="work", bufs=3) as work,
            tc.tile_pool(name="phase", bufs=GRP + 2) as phase,
        ):
            raw = data.tile([NPART, RAWF], f32, tag="raw")
            for q in range(4):
                p0, p1 = 32 * q, 32 * (q + 1)
                nc.sync.dma_start(out=raw[p0:p1, :], in_=xpc[p0:p1, :])
            sc = data.tile([NPART, H * SCW], f32, tag="sc")
            nc.sync.dma_start(out=sc[:], in_=scal[:])
            eps8 = data.tile([NPART, 1], f32, tag="eps8")
            nc.vector.memset(eps8[:], 1e-8)

            accP = data.tile([NPART, H], f32, tag="accP")
            accG = data.tile([NPART, H], f32, tag="accG")
            accS = data.tile([NPART, H], f32, tag="accS")
            accL = data.tile([NPART, H], f32, tag="accL")

            xs = [raw[:, FPT * d : FPT * (d + 1)] for d in range(3)]

            def col(h, j):
                return sc[:, SCW * h + j : SCW * h + j + 1]

            for h0 in range(0, H, GRP):
                hs = range(h0, min(h0 + GRP, H))
                tcls, gs, ps, sels = {}, {}, {}, {}
                for h in hs:
                    # pred: tcl = clip(max_d((x_d-c_d)^2 - r_d^2), +-0.2)
                    # every 3rd h runs the y/z legs on DVE to balance engines
                    if h % 3 == 2:
                        qsy = work.tile([NPART, FPT], f32, tag="qsy")
                        nc.vector._custom_dve(OPS["ANT_SQSUB"], out=qsy[:],
                                              in0=xs[1],
                                              s0=col(h, 6), s1=col(h, 2))
                        m1 = work.tile([NPART, FPT], f32, tag="m1")
                        nc.vector._custom_dve(OPS["ANT_SQSUBMAX"], out=m1[:],
                                              in0=xs[2], in1=qsy[:],
                                              s0=col(h, 7), s1=col(h, 3))
                    else:
                        sqy = work.tile([NPART, FPT], f32, tag="sqy")
                        nc.scalar.activation(sqy[:], xs[1], Act.Square,
                                             bias=col(h, 0), scale=1.0)
                        sqz = work.tile([NPART, FPT], f32, tag="sqz")
                        nc.scalar.activation(sqz[:], xs[2], Act.Square,
                                             bias=col(h, 1), scale=1.0)
                        m1 = work.tile([NPART, FPT], f32, tag="m1")
                        nc.vector._custom_dve(OPS["ANT_SUB2MAX"], out=m1[:],
                                              in0=sqy[:], in1=sqz[:],
                                              s0=col(h, 2), s1=col(h, 3))
                    tcl = phase.tile([NPART, FPT], f32, tag="tcl")
                    nc.vector._custom_dve(OPS["ANT_SQMAXCLIP"], out=tcl[:],
                                          in0=xs[0], in1=m1[:],
                                          s0=col(h, 4), s1=col(h, 5), imm2=-0.2)
                    tcls[h] = tcl

                    # gt: g = 1{max_d((x_d-c'_d)^2 - r'^2_d) < 0}
                    sqgy = work.tile([NPART, FPT], f32, tag="sqgy")
                    nc.scalar.activation(sqgy[:], xs[1], Act.Square,
                                         bias=col(h, 8), scale=1.0)
                    sqgz = work.tile([NPART, FPT], f32, tag="sqgz")
                    nc.scalar.activation(sqgz[:], xs[2], Act.Square,
                                         bias=col(h, 9), scale=1.0)
                    mg1 = work.tile([NPART, FPT], f32, tag="mg1")
                    nc.vector._custom_dve(OPS["ANT_SUB2MAX"], out=mg1[:],
                                          in0=sqgy[:], in1=sqgz[:],
                                          s0=col(h, 10), s1=col(h, 11),)
                    g = phase.tile([NPART, FPT], f32, tag="g")
                    nc.vector._custom_dve(OPS["ANT_SQMAXLT0SUM"], out=g[:],
                                          in0=xs[0], in1=mg1[:],
                                          s0=col(h, 12), s1=col(h, 13),
                                          accum_out=accG[:, h : h + 1])
                    gs[h] = g

                for h in hs:   # sigmoid phase (one ACT table load per group)
                    p = phase.tile([NPART, FPT], f32, tag="p")
                    nc.scalar.activation(p[:], tcls[h][:], Act.Sigmoid,
                                         bias=0.0, scale=-100.0,
                                         accum_out=accP[:, h : h + 1])
                    ps[h] = p
                for h in hs:   # sel = |p + (g-1)|
                    sel = phase.tile([NPART, FPT], f32, tag="sel")
                    nc.vector._custom_dve(OPS["ANT_ABSPG1SUM"], out=sel[:],
                                          in0=ps[h][:], in1=gs[h][:],
                                          accum_out=accS[:, h : h + 1])
                    sels[h] = sel
                for h in hs:   # Ln phase (one ACT table load per group)
                    lnsel = work.tile([NPART, FPT], f32, tag="lnsel")
                    nc.scalar.activation(lnsel[:], sels[h][:], Act.Ln,
                                         bias=eps8[:], scale=1.0,
                                         accum_out=accL[:, h : h + 1])

            nc.sync.dma_start(out=accP_d[:], in_=accP[:])
            nc.sync.dma_start(out=accG_d[:], in_=accG[:])
            nc.sync.dma_start(out=accS_d[:], in_=accS[:])
            nc.sync.dma_start(out=accL_d[:], in_=accL[:])

    nc.compile()
    return nc


def _get_module():
    if "nc" not in _CACHE:
        _CACHE["nc"] = _build_module()
    return _CACHE["nc"]


def _make_inputs(X_pc, y_bbvert_pred, Y_bbvert):
    """Build per-core input maps (host-side shard + scalar precompute)."""
    X_pc = np.ascontiguousarray(X_pc, dtype=np.float32)
    pred = np.asarray(y_bbvert_pred, dtype=np.float32)
    gt = np.asarray(Y_bbvert, dtype=np.float32)

    # columns per (B,H): pred [-c_y, -c_z, rsq_y, rsq_z, c_x, rsq_x, c_y, c_z]
    # then gt [-c'_y, -c'_z, rsq'_y, rsq'_z, c'_x, rsq'_x], 2 pad
    def params(t, with_pos):
        a = t[:, :, 0, :]
        b = t[:, :, 1, :]
        c = ((a + b) * np.float32(0.5)).astype(np.float32)
        r = ((b - a) * np.float32(0.5)).astype(np.float32)
        rsq = (r * r).astype(np.float32)
        cols = [-c[:, :, 1], -c[:, :, 2], rsq[:, :, 1], rsq[:, :, 2],
                c[:, :, 0], rsq[:, :, 0]]
        if with_pos:
            cols += [c[:, :, 1], c[:, :, 2]]
        return np.stack(cols, axis=-1)

    zpad = np.zeros((B, H, 2), dtype=np.float32)
    sc_all = np.concatenate([params(pred, True), params(gt, False), zpad],
                            axis=-1)  # [B,H,16]

    in_maps = []
    for k in range(NCORES):
        rows = []
        scs = []
        for b in range(BPC):
            bi = BPC * k + b
            # de-interleave xyz on host: partition row = [x(640)|y(640)|z(640)]
            xyz = X_pc[bi].reshape(PPB, FPT, 9)[:, :, :3]
            rows.append(xyz.transpose(0, 2, 1).reshape(PPB, RAWF))
            scs.append(np.broadcast_to(sc_all[bi][None], (PPB, H, SCW)))
        in_maps.append({
            "xpc": np.ascontiguousarray(np.concatenate(rows, axis=0)),
            "scal": np.ascontiguousarray(
                np.concatenate(scs, axis=0).reshape(NPART, H * SCW)),
        })
    return in_maps


def _combine(results, y_bbvert_pred, Y_bbvert):
    """Host-side: partition+core reduction and final loss combine (f64)."""
    pred = np.asarray(y_bbvert_pred, dtype=np.float32)
    gt = np.asarray(Y_bbvert, dtype=np.float32)

    Sp = np.zeros((B, H)); Sg = np.zeros((B, H))
    Ss = np.zeros((B, H)); Sl = np.zeros((B, H))
    for k in range(NCORES):
        r = results[k]
        for b in range(BPC):
            bi = BPC * k + b
            s = slice(PPB * b, PPB * (b + 1))
            Sp[bi] = r["accP"][s].astype(np.float64).sum(axis=0)
            Sg[bi] = r["accG"][s].astype(np.float64).sum(axis=0)
            Ss[bi] = r["accS"][s].astype(np.float64).sum(axis=0)
            Sl[bi] = r["accL"][s].astype(np.float64).sum(axis=0)

    Tp = (Ss + Sg + Sp - float(N)) * 0.5
    helper = (gt.reshape(B, H, 6).sum(axis=-1) > 0.0).astype(np.float64)
    Sce = -Sl

    denom_ce = helper.sum() * N
    loss_ce = (Sce * helper).sum() / denom_ce

    iou_all = -(Tp / (Sp + Sg - Tp + 1e-6))
    loss_iou = (iou_all * helper).sum() / helper.sum()

    l2_all = ((gt.astype(np.float64) - pred.astype(np.float64)) ** 2
              ).reshape(B, H, 6).mean(axis=-1)
    l2_pos = (l2_all * helper).sum() / helper.sum()
    negw = (1.0 - helper)[:, :, None]
    dneg = (pred[:, :, 0, :].astype(np.float64) - pred[:, :, 1, :].astype(np.float64))
    l2_neg = ((negw * dneg) ** 2).sum() / ((1.0 - helper).sum() + 1e-8)
    loss_l2 = l2_pos + l2_neg

    total = loss_ce + loss_l2 + loss_iou
    return (np.float32(total), np.float32(loss_l2),
            np.float32(loss_ce), np.float32(loss_iou))


def run(X_pc, y_bbvert_pred, Y_bbvert, trace=False):
    from concourse.bass_utils import run_bass_kernel_spmd

    nc = _get_module()
    in_maps = _make_inputs(X_pc, y_bbvert_pred, Y_bbvert)
    res = run_bass_kernel_spmd(nc, in_maps, core_ids=list(range(NCORES)),
                               trace=trace)
    out = _combine(res.results, y_bbvert_pred, Y_bbvert)
    return out, res


def kernel(X_pc, y_bbvert_pred, Y_bbvert):
    out, _ = run(X_pc, y_bbvert_pred, Y_bbvert, trace=False)
    return out
